# revision 1
# baseline (speedup 1.0000x reference)
"""DualEEG connectivity features on 8 Trainium2 NeuronCores (Bass/Tile).

Sharding: core c -> batch c//2, band-half c%2 (bands {0,1,2} / {3,4,5}).
One SPMD program; per-core variation lives entirely in input tensor content
(DFT tables over a per-core frequency row-list, station matrices).

Per core (batch x, 64 ch = eeg1|eeg2, T=2048):
  fwd DFT   : X[f,ch] over the core's padded 576-row frequency list (fp32)
  stations  : STK[128, 9*128] assembled from X via 4 SBUF DMAs per slot
  inv DFT   : per band-unit, psum[b|him, 2048] += STK_slot.T @ table_slot
  phases    : p = arctan(him/b) + pi*(b<0)*sign(him); pw = b^2 + him^2
  pair stage: d = p1_i - p2_j via bf16 hi/lo replication matmuls; fused
              Sign/Abs/scalar_tensor_tensor accumulates -> pli/pdiff/wpli
  grams     : PE-transposed bf16 matmuls -> plv re/im1/im2, tcorr, pcorr
  coherence : pp/(pp+1e-8) == 1.0f in fp32 for non-degenerate signals, so
              coh == (#in-band freqs)/1025 exactly (host-side constant)
Final normalization of the tiny reductions happens host-side.
"""
import numpy as np
import ml_dtypes

import concourse.bass as bass
import concourse.mybir as mybir
import concourse.tile as tile
from concourse.bass_utils import run_bass_kernel_spmd

F32 = mybir.dt.float32
BF16 = mybir.dt.bfloat16
F32R = mybir.dt.float32r
AF = mybir.ActivationFunctionType
ALU = mybir.AluOpType


def legalize_waits(nc, max_waits=1):
    """This walrus build rejects instructions with >1 semaphore wait.
    Hoist extra waits onto NoOps inserted before the instruction on the
    same engine (engine program order runs them first)."""
    ctr = 0
    n_fixed = 0
    for bb in nc.main_func.blocks:
        out = []
        changed = False
        for ins in bb.instructions:
            si = ins.sync_info
            if si is not None and si.on_wait and len(si.on_wait) > max_waits:
                waits = list(si.on_wait)
                for w in waits[:-max_waits]:
                    ctr += 1
                    nop = mybir.InstNoOp(name=f"waitfix-{ctr}", ins=[], outs=[])
                    nop.engine = ins.engine
                    nop.sync_info = mybir.SyncInfo(on_wait=[w], on_update=[])
                    out.append(nop)
                ins.sync_info = mybir.SyncInfo(
                    on_wait=waits[-max_waits:], on_update=si.on_update)
                n_fixed += 1
                changed = True
            out.append(ins)
        if changed:
            try:
                bb.instructions = out
            except Exception:
                li = bb.instructions
                li.clear()
                li.extend(out)
    return n_fixed


FS = 256
T = 2048
C = 32
EPS = 1e-8
BANDS = [(0.5, 45.0), (0.5, 4.0), (4.0, 8.0), (8.0, 13.0), (13.0, 30.0),
         (30.0, 45.0)]
NRFFT = T // 2 + 1  # 1025

NSLOT = 9                     # inv-DFT chunk slots (64 freqs each)
U_SLOTS = [(0, 6), (6, 8), (8, 9)]  # unit -> slot range
RL = NSLOT * 64               # padded row-list length = 576
NTC = T // 128                # 16 t-chunks
NFC = 5                       # fwd f-chunks: 4x128 + 1x64
NPT = 8                       # pair tiles (1024 pairs / 128)
NHT = 16                      # pair half-tiles (t split into 2x1024)
PI = float(np.float32(np.pi))

# unit -> band per half (band slot needs: b0:6 b1:1 b2:1 | b4:3 b5:2 b3:1)
UNIT_BANDS = [[0, 1, 2], [4, 5, 3]]


def band_rows(bi):
    freqs = np.fft.rfftfreq(T, d=1.0 / FS)
    lo, hi = BANDS[bi]
    return np.where((freqs >= lo) & (freqs <= hi))[0]


def rowlist(half):
    rows = np.full(RL, -1, np.int64)
    for u, bi in enumerate(UNIT_BANDS[half]):
        s0, _s1 = U_SLOTS[u]
        r = band_rows(bi)
        rows[s0 * 64: s0 * 64 + len(r)] = r
    return rows


def _bf16(x):
    return np.asarray(x, np.float32).astype(ml_dtypes.bfloat16)


def host_tables(half):
    """fwd wtc/wts [128, NTC*RL] and inv tbl [128, NSLOT*T], all fp32."""
    rows = rowlist(half)
    t = np.arange(T)
    valid = rows >= 0
    rr = np.where(valid, rows, 0)
    ang = 2 * np.pi * np.outer(rr, t) / T          # [RL, T]
    cos_ft = (np.cos(ang) * valid[:, None]).astype(np.float32)
    sin_ft = (np.sin(ang) * valid[:, None]).astype(np.float32)
    # fc-major: block fc holds [128t, NTC*128] with 128-wide slots per tcb
    wtc = np.zeros((128, NFC * NTC * 128), np.float32)
    wts = np.zeros((128, NFC * NTC * 128), np.float32)
    for fc in range(NFC):
        nf = 128 if fc < 4 else 64
        for tc in range(NTC):
            c0 = (fc * NTC + tc) * 128
            blk_c = cos_ft[fc * 128:fc * 128 + nf,
                           tc * 128:(tc + 1) * 128].T
            blk_s = -sin_ft[fc * 128:fc * 128 + nf,
                            tc * 128:(tc + 1) * 128].T
            wtc[:, c0:c0 + nf] = blk_c
            wts[:, c0:c0 + nf] = blk_s
    # inv table: slot rows 0:64 = cos(f_r, t), 64:128 = sin(f_r, t)
    tbl = np.zeros((128, NSLOT * T), np.float32)
    for s in range(NSLOT):
        tbl[0:64, s * T:(s + 1) * T] = cos_ft[s * 64:(s + 1) * 64]
        tbl[64:128, s * T:(s + 1) * T] = sin_ft[s * 64:(s + 1) * 64]
    return wtc, wts, tbl


def host_stations():
    dst = np.zeros((128, NPT * 128), np.float32)
    pwst = np.zeros((64, NPT * 128), np.float32)
    for r in range(NPT):
        for m in range(128):
            P = r * 128 + m
            i, j = P // 32, P % 32
            col = r * 128 + m
            dst[i, col] = 1.0        # p1h
            dst[32 + j, col] = -1.0  # p2h
            dst[64 + i, col] = 1.0   # p1l
            dst[96 + j, col] = -1.0  # p2l
            pwst[i, col] = 1.0       # pw1
            pwst[32 + j, col] = 1.0  # pw2
    return _bf16(dst), _bf16(pwst)


def register_const(nc, value, dtype=F32):
    key = (dtype, float(value))
    if key not in nc.const_aps.aps:
        tns = nc.alloc_sbuf_tensor(f"const-{dtype.name}-{value}", [128, 1], dtype)
        nc.gpsimd.memset(tns.ap(), value)
        nc.const_aps.aps[key] = tns.ap()
    return nc.const_aps.aps[key]


def build_nc():
    nc = bass.Bass()
    for v in (PI, -1.0, 2.0, 0.5, 1.5):
        register_const(nc, v)

    xt = nc.dram_tensor("xt", [128, NTC * 64], F32, kind="ExternalInput")
    wtcD = nc.dram_tensor("wtc", [128, NFC * NTC * 128], F32,
                          kind="ExternalInput")
    wtsD = nc.dram_tensor("wts", [128, NFC * NTC * 128], F32,
                          kind="ExternalInput")
    tblD = nc.dram_tensor("tbl", [128, NSLOT * T], F32, kind="ExternalInput")
    dstD = nc.dram_tensor("dst", [128, NPT * 128], BF16, kind="ExternalInput")
    pwstD = nc.dram_tensor("pwst", [64, NPT * 128], BF16, kind="ExternalInput")
    identD = nc.dram_tensor("ident", [128, 128], BF16, kind="ExternalInput")

    o_gram = nc.dram_tensor("o_gram", [32, 3 * 160], F32, kind="ExternalOutput")
    o_pli = nc.dram_tensor("o_pli", [128, 3 * NHT], F32, kind="ExternalOutput")
    o_wp = nc.dram_tensor("o_wp", [128, 3 * NHT], F32, kind="ExternalOutput")
    o_ab = nc.dram_tensor("o_ab", [128, 3 * NHT], F32, kind="ExternalOutput")
    o_st = nc.dram_tensor("o_st", [64, 3 * 4], F32, kind="ExternalOutput")


    with tile.TileContext(nc) as tc:
        with (
            tc.tile_pool(name="cst", bufs=1) as cst,
            tc.tile_pool(name="stream", bufs=2) as stream,
            tc.tile_pool(name="fstream", bufs=1) as fstream,
            tc.tile_pool(name="unit", bufs=2) as up,
            tc.tile_pool(name="scr", bufs=1) as scr,
            tc.tile_pool(name="pscr", bufs=2) as pscr,
            tc.tile_pool(name="acc", bufs=1) as accp,
        ):
            # ------- constants in -------
            xt_sb = cst.tile([128, NTC * 64], F32, tag="xt")
            nc.sync.dma_start(xt_sb[:], xt[:])
            dst_sb = cst.tile([128, NPT * 128], BF16, tag="dst")
            nc.sync.dma_start(dst_sb[:], dstD[:])
            pwst_sb = cst.tile([64, NPT * 128], BF16, tag="pwst")
            nc.sync.dma_start(pwst_sb[:], pwstD[:])
            ident_sb = cst.tile([128, 128], BF16, tag="ident")
            nc.sync.dma_start(ident_sb[:], identD[:])

            # ------- forward DFT over padded row-list -------
            x_sb = cst.tile([128, NFC * 128], F32, tag="xsb")    # re|im per fc
            xn_sb = cst.tile([128, NFC * 128], F32, tag="xnsb")  # -im half
            with tc.tile_pool(name="fwdp", bufs=2, space="PSUM") as fp:
                for fc in range(NFC):
                    nf = 128 if fc < 4 else 64
                    psr = fp.tile([128, 64], F32, tag="fpsr")
                    psi = fp.tile([128, 64], F32, tag="fpsi")
                    wcs = fstream.tile([128, NTC * 128], F32, tag="wcs")
                    wss = fstream.tile([128, NTC * 128], F32, tag="wss")
                    fb = fc * NTC * 128
                    nc.sync.dma_start(wcs[:], wtcD[:, fb:fb + NTC * 128])
                    nc.sync.dma_start(wss[:], wtsD[:, fb:fb + NTC * 128])
                    for tcb in range(NTC):
                        first, last = tcb == 0, tcb == NTC - 1
                        xtb = xt_sb[:, tcb * 64:(tcb + 1) * 64]
                        nc.tensor.matmul(
                            psr[0:nf, :],
                            lhsT=wcs[:, tcb * 128:tcb * 128 + nf],
                            rhs=xtb, start=first, stop=last)
                        nc.tensor.matmul(
                            psi[0:nf, :],
                            lhsT=wss[:, tcb * 128:tcb * 128 + nf],
                            rhs=xtb, start=first, stop=last)
                    nc.scalar.copy(x_sb[0:nf, fc * 128:fc * 128 + 64],
                                   psr[0:nf, :])
                    nc.scalar.copy(x_sb[0:nf, fc * 128 + 64:(fc + 1) * 128],
                                   psi[0:nf, :])
                    nc.vector.tensor_scalar_mul(
                        xn_sb[0:nf, fc * 128 + 64:(fc + 1) * 128],
                        psi[0:nf, :], -1.0)

            # ------- station assembly: STK rows 0:64=cos, 64:128=sin -------
            stk = cst.tile([128, NSLOT * 128], F32, tag="stk")
            for s in range(NSLOT):
                fcb, sub = divmod(s, 2)
                rsl = slice(sub * 64, (sub + 1) * 64)
                re_src = x_sb[rsl, fcb * 128 + 0:fcb * 128 + 64]
                im_src = x_sb[rsl, fcb * 128 + 64:fcb * 128 + 128]
                imn_src = xn_sb[rsl, fcb * 128 + 64:fcb * 128 + 128]
                nc.sync.dma_start(stk[0:64, s * 128 + 0:s * 128 + 64], re_src)
                nc.sync.dma_start(stk[0:64, s * 128 + 64:s * 128 + 128], im_src)
                nc.sync.dma_start(stk[64:128, s * 128 + 0:s * 128 + 64], imn_src)
                nc.sync.dma_start(stk[64:128, s * 128 + 64:s * 128 + 128], re_src)

            # ------- accumulators -------
            acc_pli = accp.tile([128, 3 * NHT], F32, tag="apli")
            acc_wp = accp.tile([128, 3 * NHT], F32, tag="awp")
            acc_ab = accp.tile([128, 3 * NHT], F32, tag="aab")
            stats = accp.tile([64, 3 * 4], F32, tag="stats")
            gram_sb = accp.tile([32, 3 * 160], F32, tag="gram")

            for u in range(3):
                s0, s1 = U_SLOTS[u]
                # ---- inverse DFT ----
                bh = up.tile([64, 2 * T], F32, tag="bh")
                with tc.tile_pool(name=f"invp{u}", bufs=1, space="PSUM") as ivp:
                    ps = ivp.tile([128, T], F32, tag="ivps")
                    for s in range(s0, s1):
                        tb = stream.tile([128, T], F32, tag="tb")
                        nc.sync.dma_start(tb[:], tblD[:, s * T:(s + 1) * T])
                        for ns in range(4):
                            sl = slice(ns * 512, (ns + 1) * 512)
                            nc.tensor.matmul(
                                ps[:, sl],
                                lhsT=stk[:, s * 128:(s + 1) * 128],
                                rhs=tb[:, sl],
                                start=(s == s0), stop=(s == s1 - 1))
                    nc.scalar.copy(bh[:, 0:T], ps[0:64, :])
                    hstage = scr.tile([128, T], F32, tag="sD")
                    nc.vector.tensor_copy(hstage[64:128, :], ps[64:128, :])
                    nc.sync.dma_start(bh[:, T:2 * T], hstage[64:128, :])
                b_ap = bh[:, 0:T]
                h_ap = bh[:, T:2 * T]


                # ---- phases / powers (all on partitions 0:64) ----
                rb = scr.tile([64, T], F32, tag="sA")
                nc.vector.reciprocal(rb[:], b_ap)
                ratio = scr.tile([64, T], F32, tag="sB")
                nc.vector.tensor_tensor(ratio[:], h_ap, rb[:], ALU.mult)
                at = scr.tile([64, T], F32, tag="sA")
                nc.scalar.activation(at[:], ratio[:], AF.Arctan)
                sgnh = scr.tile([64, T], F32, tag="sC")
                nc.scalar.activation(sgnh[:], h_ap, AF.Sign)
                corr = scr.tile([64, T], F32, tag="sB")
                nc.vector.scalar_tensor_tensor(
                    out=corr[:], in0=b_ap, scalar=0.0, in1=sgnh[:],
                    op0=ALU.is_lt, op1=ALU.mult)
                p = scr.tile([64, T], F32, tag="sP")
                nc.vector.scalar_tensor_tensor(
                    out=p[:], in0=corr[:], scalar=PI, op0=ALU.mult,
                    op1=ALU.add, in1=at[:])
                # d-matmul moving operand: rows [p1h|p2h ; p1l|p2l]
                phl = up.tile([128, T], BF16, tag="phl")
                nc.gpsimd.tensor_copy(phl[0:64, :], p[:])
                pl64 = scr.tile([64, T], BF16, tag="pl64")
                nc.vector.tensor_tensor(pl64[:], p[:], phl[0:64, :],
                                        ALU.subtract)
                nc.sync.dma_start(phl[64:128, :], pl64[:])
                # pw = b^2 + him^2 (raw scale)
                sqb = scr.tile([64, 2 * T], F32, tag="sD")
                nc.scalar.activation(sqb[:], bh[:], AF.Square)
                pwf = scr.tile([64, T], F32, tag="sE")
                nc.vector.tensor_tensor(pwf[:], sqb[:, 0:T], sqb[:, T:2 * T],
                                        ALU.add)
                pwb = up.tile([64, T], BF16, tag="pwb")
                nc.gpsimd.tensor_copy(pwb[:], pwf[:])
                # stats: [Sb, Sb2, Spw, Spw2]
                junk = scr.tile([64, T], BF16, tag="junk")
                nc.scalar.activation(junk[:], b_ap, AF.Copy,
                                     accum_out=stats[:, u * 4:u * 4 + 1])
                nc.scalar.activation(junk[:], b_ap, AF.Square,
                                     accum_out=stats[:, u * 4 + 1:u * 4 + 2])
                nc.scalar.activation(junk[:], pwf[:], AF.Copy,
                                     accum_out=stats[:, u * 4 + 2:u * 4 + 3])
                nc.scalar.activation(junk[:], pwf[:], AF.Square,
                                     accum_out=stats[:, u * 4 + 3:u * 4 + 4])
                # c|s pack: rows 0:64 = cos(p), 64:128 = sin(p)
                cs = up.tile([128, T], BF16, tag="cs")
                s64 = scr.tile([64, T], BF16, tag="s64")
                nc.scalar.activation(s64[:], p[:], AF.Sin)
                nc.sync.dma_start(cs[64:128, :], s64[:])
                sh = scr.tile([64, T], F32, tag="sC")
                nc.scalar.activation(sh[:], p[:], AF.Sin, scale=0.5)
                csq = scr.tile([64, T], F32, tag="sB")
                nc.scalar.activation(csq[:], sh[:], AF.Square)
                nc.vector.tensor_scalar(cs[0:64, :], csq[:], -2.0, 1.0,
                                        op0=ALU.mult, op1=ALU.add)
                # b|pw pack for tcorr/pcorr grams
                gp = up.tile([128, T], BF16, tag="gp")
                nc.gpsimd.tensor_copy(gp[0:64, :], b_ap)
                nc.sync.dma_start(gp[64:128, :], pwb[:])

                # ---- transposes + grams ----
                csT = scr.tile([128, T], BF16, tag="csT")
                gpT = scr.tile([128, T], BF16, tag="gpT")
                with (
                    tc.tile_pool(name=f"grp{u}", bufs=3, space="PSUM") as gpp,
                    tc.tile_pool(name=f"grg{u}", bufs=1, space="PSUM") as ggp,
                ):
                    for tau in range(NTC):
                        sl = slice(tau * 128, (tau + 1) * 128)
                        pt1 = gpp.tile([128, 128], BF16, tag="tps")
                        nc.tensor.transpose(pt1[:], cs[:, sl], ident_sb[:])
                        nc.vector.tensor_copy(csT[:, sl], pt1[:])
                        pt2 = gpp.tile([128, 128], BF16, tag="tps")
                        nc.tensor.transpose(pt2[:], gp[:, sl], ident_sb[:])
                        nc.vector.tensor_copy(gpT[:, sl], pt2[:])
                    g_re = ggp.tile([32, 32], F32, tag="g0")
                    g_im1 = ggp.tile([32, 32], F32, tag="g1")
                    g_im2 = ggp.tile([32, 32], F32, tag="g2")
                    g_tc = ggp.tile([32, 32], F32, tag="g3")
                    g_pc = ggp.tile([32, 32], F32, tag="g4")
                    for tau in range(NTC):
                        st_, sp_ = tau == 0, tau == NTC - 1
                        def cblk(a, b_):
                            return csT[:, tau * 128 + a:tau * 128 + b_]
                        def gblk(a, b_):
                            return gpT[:, tau * 128 + a:tau * 128 + b_]
                        # plv re = c1.c2 + s1.s2
                        nc.tensor.matmul(g_re[:], lhsT=cblk(0, 32),
                                         rhs=cblk(32, 64), start=st_,
                                         stop=False)
                        nc.tensor.matmul(g_re[:], lhsT=cblk(64, 96),
                                         rhs=cblk(96, 128), start=False,
                                         stop=sp_)
                        nc.tensor.matmul(g_im1[:], lhsT=cblk(64, 96),
                                         rhs=cblk(32, 64), start=st_, stop=sp_)
                        nc.tensor.matmul(g_im2[:], lhsT=cblk(0, 32),
                                         rhs=cblk(96, 128), start=st_,
                                         stop=sp_)
                        nc.tensor.matmul(g_tc[:], lhsT=gblk(0, 32),
                                         rhs=gblk(32, 64), start=st_, stop=sp_)
                        nc.tensor.matmul(g_pc[:], lhsT=gblk(64, 96),
                                         rhs=gblk(96, 128), start=st_,
                                         stop=sp_)
                    for qi, gq in enumerate((g_re, g_im1, g_im2, g_tc, g_pc)):
                        nc.scalar.copy(
                            gram_sb[:, u * 160 + qi * 32:u * 160 + (qi + 1) * 32],
                            gq[:])

                # ---- pair stage ----
                with (
                    tc.tile_pool(name=f"prp{u}", bufs=2, space="PSUM") as prp,
                    tc.tile_pool(name=f"prq{u}", bufs=1, space="PSUM") as prp2,
                ):
                    for ht in range(NHT):
                        r, th = divmod(ht, 2)
                        dps = prp.tile([128, 1024], F32, tag="dps")
                        pps = prp2.tile([128, 1024], F32, tag="pps")
                        for ns in range(2):
                            msl = slice(th * 1024 + ns * 512,
                                        th * 1024 + ns * 512 + 512)
                            osl = slice(ns * 512, ns * 512 + 512)
                            nc.tensor.matmul(
                                dps[:, osl],
                                lhsT=dst_sb[:, r * 128:(r + 1) * 128],
                                rhs=phl[:, msl], start=True, stop=True)
                            nc.tensor.matmul(
                                pps[:, osl],
                                lhsT=pwst_sb[:, r * 128:(r + 1) * 128],
                                rhs=pwb[:, msl], start=True, stop=True)
                        k = u * NHT + ht
                        sgn = pscr.tile([128, 1024], BF16, tag="sgn")
                        nc.scalar.activation(sgn[:], dps[:], AF.Sign,
                                             accum_out=acc_pli[:, k:k + 1])
                        wsc = pscr.tile([128, 1024], BF16, tag="wsc")
                        nc.vector.scalar_tensor_tensor(
                            out=wsc[:], in0=pps[:], scalar=1.0, in1=sgn[:],
                            op0=ALU.mult, op1=ALU.mult,
                            accum_out=acc_wp[:, k:k + 1])
                        asc = pscr.tile([128, 1024], BF16, tag="asc")
                        if ht % 2 == 0:
                            nc.scalar.activation(asc[:], dps[:], AF.Abs,
                                                 accum_out=acc_ab[:, k:k + 1])
                        else:
                            nc.vector.scalar_tensor_tensor(
                                out=asc[:], in0=dps[:], scalar=1.0,
                                in1=sgn[:], op0=ALU.mult, op1=ALU.mult,
                                accum_out=acc_ab[:, k:k + 1])

            # ------- outputs -------
            nc.sync.dma_start(o_pli[:], acc_pli[:])
            nc.sync.dma_start(o_wp[:], acc_wp[:])
            nc.sync.dma_start(o_ab[:], acc_ab[:])
            nc.sync.dma_start(o_st[:], stats[:])
            nc.sync.dma_start(o_gram[:], gram_sb[:])

    legalize_waits(nc)
    return nc


_CACHE = {}


def _get_nc():
    if "nc" not in _CACHE:
        _CACHE["nc"] = build_nc()
    return _CACHE["nc"]


def _core_inputs(half, x64):
    if ("tab", half) not in _CACHE:
        _CACHE[("tab", half)] = host_tables(half)
    wtc, wts, tbl = _CACHE[("tab", half)]
    if "st" not in _CACHE:
        _CACHE["st"] = host_stations()
        _CACHE["ident"] = _bf16(np.eye(128, dtype=np.float32))
    dst, pwst = _CACHE["st"]
    xt = np.zeros((128, NTC * 64), np.float32)
    for tcb in range(NTC):
        xt[:, tcb * 64:(tcb + 1) * 64] = x64[:, tcb * 128:(tcb + 1) * 128].T
    return {
        "xt": xt, "wtc": wtc, "wts": wts, "tbl": tbl,
        "dst": dst, "pwst": pwst, "ident": _CACHE["ident"],
    }


def _finalize(res_core, half):
    """res_core: dict of per-core outputs -> [3 bands][7, C, C] features."""
    out = {}
    scale2 = (T / 2.0) ** 2
    for u, bi in enumerate(UNIT_BANDS[half]):
        g = res_core["o_gram"][:, u * 160:(u + 1) * 160]
        re, im1, im2 = g[:, 0:32], g[:, 32:64], g[:, 64:96]
        Gb, Gpw = g[:, 96:128], g[:, 128:160]
        plv = np.sqrt(re * re + (im1 - im2) ** 2) / T

        st = res_core["o_st"][:, u * 4:(u + 1) * 4]  # [64ch, 4]
        Sb, Sb2, Spw, Spw2 = st.T

        def pair_sum(a):
            # a: [128, NHT] -> [1024] pair sums -> [32, 32]
            cols = a[:, u * NHT:(u + 1) * NHT]
            s = np.zeros(1024, np.float64)
            for ht in range(NHT):
                r = ht // 2
                s[r * 128:(r + 1) * 128] += cols[:, ht]
            return s.reshape(32, 32)

        Spli = pair_sum(res_core["o_pli"])
        Swp = pair_sum(res_core["o_wp"])
        Sab = pair_sum(res_core["o_ab"])
        pli = np.abs(Spli) / T
        pdiff = Sab / T
        den = Spw[:C, None] + Spw[C:, None].T + 2 * EPS * scale2
        wpli = np.abs(Swp) / den

        def corr(G, S, S2):
            mu = S / T
            var = (S2 - T * mu * mu) / (T - 1)
            sd = np.sqrt(np.maximum(var, 0))
            N = G - T * np.outer(mu[:C], mu[C:])
            return N / (np.outer(sd[:C] + EPS * np.sqrt(scale2) ** 0,
                                 sd[C:]) * T + 1e-300)

        # tcorr/pcorr: scale cancels between N and sd1*sd2; EPS negligible
        def corr2(G, S, S2):
            mu = S / T
            var = (S2 - T * mu * mu) / (T - 1)
            sd = np.sqrt(np.maximum(var, 0))
            N = G - T * np.outer(mu[:C], mu[C:])
            return N / ((np.outer(sd[:C], sd[C:]) + 1e-300) * T)

        tcorr = corr2(Gb, Sb, Sb2)
        pcorr = corr2(Gpw, Spw, Spw2)
        coh = np.full((C, C), len(band_rows(bi)) / NRFFT, np.float32)

        out[bi] = np.stack([plv, pli, wpli, coh, pcorr, pdiff,
                            tcorr]).astype(np.float32)
    return out


def kernel(eeg1, eeg2):
    eeg1 = np.asarray(eeg1, np.float32)
    eeg2 = np.asarray(eeg2, np.float32)
    B = eeg1.shape[0]
    nc = _get_nc()
    in_maps = []
    for c in range(8):
        b, half = c // 2, c % 2
        x64 = np.concatenate([eeg1[b], eeg2[b]], 0)
        in_maps.append(_core_inputs(half, x64))
    res = run_bass_kernel_spmd(nc, in_maps, core_ids=list(range(8)))
    out = np.zeros((B, 6, 7, C, C), np.float32)
    for c in range(8):
        b, half = c // 2, c % 2
        feats = _finalize(res.results[c], half)
        for bi, f in feats.items():
            out[b, bi] = f
    return out

